# revision 9
# baseline (speedup 1.0000x reference)
"""Trainium2 Bass kernel for pair-masked causal self-attention.

Problem: B=4, T=2048, C=1024, H=16 heads (hd=64), GPT-style CausalSelfAttention
with a modified causal mask (odd query i cannot attend to i-1).

Sharding: 8 cores = 4 batches x 2 head-groups (8 heads each). No collectives:
each core computes a partial c_proj output (its 512 channels of y), partials
are summed pairwise on the host.

Per-core layout trick: q,k are computed TRANSPOSED [c_out, t] (channel on
partitions) so the scores matmul produces ST[k_pos, q_pos] directly, and v is
computed in natural [t, c] orientation so the AV matmul (lhsT = [v | ones])
yields y-numerator rows AND the softmax denominator rows in one pass.
The pair mask only touches diagonal 128x512 score tiles -> 4 static masks.
"""

import numpy as np
import ml_dtypes

import concourse.bass as bass
import concourse.bacc as bacc
import concourse.tile as tile
from concourse import mybir
from concourse.bass_utils import run_bass_kernel_spmd

B, T, C, H = 4, 2048, 1024, 16
HD = C // H          # 64
G = 8                # cores
HPC = 8              # heads per core
PAIRS = HPC // 2     # head pairs per core
NT = T // 512        # 4 q/t column blocks of 512
KT = T // 128        # 16 k row tiles of 128
CT = C // 128        # 8 c_in tiles

DT_NAME = "float32r"   # "float32r" | "bfloat16"

_cache = {}


def _dt(dt_name):
    return getattr(mybir.dt, dt_name)


def _np_dt(dt_name):
    return np.float32 if dt_name == "float32r" else ml_dtypes.bfloat16


def build_nc(dt_name=DT_NAME):
    """Build (and cache) the per-core Bass program."""
    if dt_name in _cache:
        return _cache[dt_name]

    DT = _dt(dt_name)
    F32 = mybir.dt.float32
    nc = bacc.Bacc()

    xT_d = nc.declare_dram_parameter("xT", [C, T], DT, isOutput=False)
    wqk_d = nc.declare_dram_parameter("wqkT", [C, 1024], DT, isOutput=False)
    wv_d = nc.declare_dram_parameter("wvT", [C, 512], DT, isOutput=False)
    wp_d = nc.declare_dram_parameter("wprojT", [512, C], DT, isOutput=False)
    mask_d = nc.declare_dram_parameter("masks", [4, 128, 1024], DT, isOutput=False)
    bqk_d = nc.declare_dram_parameter("bqk", [128, 8], F32, isOutput=False)
    bv_d = nc.declare_dram_parameter("bv", [1, 512], DT, isOutput=False)
    out_d = nc.declare_dram_parameter("out", [C, T], F32, isOutput=True)

    EXP = mybir.ActivationFunctionType.Exp
    SCALE = 1.0 / float(np.sqrt(HD))

    with tile.TileContext(nc) as tc:
        with (
            tc.tile_pool(name="persist", bufs=1) as persist,
            tc.tile_pool(name="xw", bufs=1) as xw,
            tc.tile_pool(name="attnc", bufs=1) as attnc,
            tc.tile_pool(name="es_p", bufs=6) as es_p,
            tc.tile_pool(name="norm_p", bufs=3) as norm_p,
            tc.tile_pool(name="stage_p", bufs=4) as stage_p,
            tc.tile_pool(name="mm_ps", bufs=2, space="PSUM") as mm_ps,
            tc.tile_pool(name="st_ps", bufs=2, space="PSUM") as st_ps,
            tc.tile_pool(name="y_ps", bufs=2, space="PSUM") as y_ps,
        ):
            # ---- persistent tensors ----
            qkT = [persist.tile([128, T], DT, tag=f"qkT{m}", name=f"qkT{m}") for m in range(8)]
            v_aug = [persist.tile([128, PAIRS, 192], DT, tag=f"va{t}", name=f"va{t}") for t in range(KT)]
            yT = [persist.tile([128, T], DT, tag=f"yT{p}", name=f"yT{p}") for p in range(PAIRS)]

            # ---- input tiles ----
            xT = [xw.tile([128, T], DT, tag=f"xT{i}", name=f"xT{i}") for i in range(CT)]
            wqk = [xw.tile([128, 1024], DT, tag=f"wqk{i}", name=f"wqk{i}") for i in range(CT)]
            wv = [xw.tile([128, 512], DT, tag=f"wv{i}", name=f"wv{i}") for i in range(CT)]
            wp = [attnc.tile([128, C], DT, tag=f"wp{i}", name=f"wp{i}") for i in range(4)]
            msk = [attnc.tile([128, 2, 512], DT, tag=f"msk{d}", name=f"msk{d}") for d in range(4)]
            bqk = attnc.tile([128, 8], F32, tag="bqk")
            bv = attnc.tile([1, 512], DT, tag="bv")
            ones_r = attnc.tile([1, 128], DT, tag="ones")

            nc.sync.dma_start(bqk, bqk_d[:])
            nc.sync.dma_start(bv, bv_d[:])
            nc.vector.memset(ones_r, 1.0)
            # column-sliced xT loads: block 0 only gates on its own slice
            for i in range(CT):
                nc.sync.dma_start(wqk[i], wqk_d[128 * i:128 * i + 128, :])
                nc.sync.dma_start(xT[i][:, 0:512], xT_d[128 * i:128 * i + 128, 0:512])
            for i in range(CT):
                nc.sync.dma_start(wv[i], wv_d[128 * i:128 * i + 128, :])
            for d in range(4):
                nc.sync.dma_start(
                    msk[d], mask_d[d].rearrange("p (h q) -> p h q", h=2))
            for n2 in range(1, NT):
                for i in range(CT):
                    nc.sync.dma_start(xT[i][:, 512 * n2:512 * n2 + 512],
                                      xT_d[128 * i:128 * i + 128, 512 * n2:512 * n2 + 512])
                if n2 == 1:
                    for i in range(4):
                        nc.sync.dma_start(wp[i], wp_d[128 * i:128 * i + 128, :])

            # ---- software-pipelined emission ----
            # Tile produces a static per-engine order, so PE stalls unless
            # independent matmuls are woven between dependent ST->exp->AV
            # chains. Streams: qkv block n || attention j=n-1 || proj j=n-2.

            def qkv_groups(n):
                tsl = bass.ts(n, 512)
                groups = []
                for m in range(8):
                    def g(m=m, tsl=tsl):
                        ps = mm_ps.tile([128, 512], F32, tag="mm", name=f"mm_qk_{n}_{m}")
                        for kc in range(CT):
                            nc.tensor.matmul(
                                ps, wqk[kc][:, 128 * m:128 * m + 128], xT[kc][:, tsl],
                                start=(kc == 0), stop=(kc == CT - 1))
                        nc.vector.tensor_scalar_add(qkT[m][:, tsl], ps, bqk[:, m:m + 1])
                    groups.append(g)
                for tt in range(4 * n, 4 * n + 4):
                    def g(tt=tt):
                        ps = mm_ps.tile([128, 512], F32, tag="mm", name=f"mm_v_{tt}")
                        for kc in range(CT):
                            nc.tensor.matmul(
                                ps, xT[kc][:, 128 * tt:128 * tt + 128], wv[kc],
                                start=(kc == 0), stop=False)
                        nc.tensor.matmul(ps, ones_r, bv, start=False, stop=True,
                                         skip_group_check=True)
                        psv = ps.rearrange("p (pr two d) -> p pr two d", pr=PAIRS, two=2)
                        nc.vector.memset(v_aug[tt][:, :, 64:128], 1.0)
                        nc.vector.tensor_copy(v_aug[tt][:, :, 0:64], psv[:, :, 0, :])
                        nc.vector.tensor_copy(v_aug[tt][:, :, 128:192], psv[:, :, 1, :])
                    groups.append(g)
                return groups

            def attn_units(j):
                qsl0 = 512 * j
                kk_hi = 4 * j + 4
                units = []
                ys = {}

                def mk_kk(p, kk):
                    def u():
                        if kk == 0:
                            ys[p] = (y_ps.tile([128, 512], F32, tag="y", name=f"yA{j}_{p}"),
                                     y_ps.tile([128, 512], F32, tag="y", name=f"yB{j}_{p}"))
                        yA, yB = ys[p]
                        d = kk - 4 * j
                        q0 = 128 * d if d >= 0 else 0
                        st = st_ps.tile([128, 2, 512], F32, tag="st", name=f"st{j}_{p}_{kk}")
                        kT_t = qkT[4 + p]
                        qT_t = qkT[p]
                        ksl = bass.ts(kk, 128)
                        qsl = bass.ds(qsl0 + q0, 512 - q0)
                        nc.tensor.matmul(st[:, 0, q0:512],
                                         kT_t[0:64, ksl], qT_t[0:64, qsl],
                                         start=True, stop=True)
                        nc.tensor.matmul(st[:, 1, q0:512],
                                         kT_t[64:128, ksl], qT_t[64:128, qsl],
                                         start=True, stop=True)
                        es = es_p.tile([128, 2, 512], DT, tag="es", name=f"es{j}_{p}_{kk}")
                        nc.scalar.activation(es[:, :, q0:512], st[:, :, q0:512],
                                             EXP, scale=SCALE)
                        if d >= 0:
                            nc.vector.tensor_mul(es[:, :, q0:512], es[:, :, q0:512],
                                                 msk[d][:, :, q0:512])
                        nc.tensor.matmul(yA[:, q0:512], v_aug[kk][:, p, 0:128],
                                         es[:, 0, q0:512],
                                         start=(kk == 0), stop=(kk == kk_hi - 1),
                                         skip_group_check=True)
                        nc.tensor.matmul(yB[:, q0:512], v_aug[kk][:, p, 64:192],
                                         es[:, 1, q0:512],
                                         start=(kk == 0), stop=(kk == kk_hi - 1),
                                         skip_group_check=True)
                    return u

                def mk_norm(p):
                    def u():
                        yA, yB = ys[p]
                        tsl2 = bass.ds(qsl0, 512)
                        denA = norm_p.tile([64, 512], F32, tag="den", name=f"denA{j}_{p}")
                        nc.vector.tensor_copy(denA, yA[64:128, :])
                        recA = norm_p.tile([64, 512], F32, tag="rec", name=f"recA{j}_{p}")
                        nc.vector.reciprocal_approx_fast(out=recA, in_=denA)
                        nc.vector.tensor_mul(yT[p][0:64, tsl2], yA[0:64, :], recA)
                        denB = norm_p.tile([64, 512], F32, tag="den", name=f"denB{j}_{p}")
                        nc.vector.tensor_copy(denB, yB[0:64, :])
                        recB = norm_p.tile([64, 512], F32, tag="rec", name=f"recB{j}_{p}")
                        nc.vector.reciprocal_approx_fast(out=recB, in_=denB)
                        numB = norm_p.tile([64, 512], F32, tag="num", name=f"numB{j}_{p}")
                        nc.vector.tensor_copy(numB, yB[64:128, :])
                        nc.vector.tensor_mul(yT[p][64:128, tsl2], numB, recB)
                    return u

                for p in range(PAIRS):
                    for kk in range(kk_hi):
                        units.append(mk_kk(p, kk))
                    units.append(mk_norm(p))
                return units

            def proj_groups(j):
                qsl0 = 512 * j
                groups = []
                for o in range(8):
                    def g(o=o):
                        pp = mm_ps.tile([128, 512], F32, tag="mm", name=f"mm_pj_{j}_{o}")
                        for cpt in range(4):
                            nc.tensor.matmul(pp, wp[cpt][:, 128 * o:128 * o + 128],
                                             yT[cpt][:, bass.ds(qsl0, 512)],
                                             start=(cpt == 0), stop=(cpt == 3))
                        stg = stage_p.tile([128, 512], F32, tag="stg", name=f"stg{j}_{o}")
                        nc.vector.tensor_copy(stg, pp)
                        nc.sync.dma_start(
                            out_d[128 * o:128 * o + 128, qsl0:qsl0 + 512], stg)
                    groups.append(g)
                return groups

            def weave(*streams):
                streams = [list(st_) for st_ in streams if st_]
                order = []
                for si, st_ in enumerate(streams):
                    for i, fn in enumerate(st_):
                        order.append(((i + 0.5) / len(st_), si, i, fn))
                order.sort(key=lambda t: (t[0], t[1]))
                for _, _, _, fn in order:
                    fn()

            for n in range(NT + 2):
                weave(
                    qkv_groups(n) if n < NT else [],
                    attn_units(n - 1) if 1 <= n <= NT else [],
                    proj_groups(n - 2) if 2 <= n <= NT + 1 else [],
                )

    nc.compile()
    _cache[dt_name] = nc
    return nc


def make_masks(dt_name=DT_NAME):
    np_dt = _np_dt(dt_name)
    kk = np.arange(128)[:, None]
    qq = np.arange(512)[None, :]
    masks = np.zeros((4, 128, 1024), dtype=np_dt)
    for d in range(4):
        r = qq - kk - 128 * d
        m = (r >= 0) & ~((r == 1) & (qq % 2 == 1))
        masks[d, :, 0:512] = m.astype(np_dt)
        masks[d, :, 512:1024] = m.astype(np_dt)
    return masks


def prep_inputs(x, w_attn, b_attn, w_proj, dt_name=DT_NAME):
    np_dt = _np_dt(dt_name)
    x = np.asarray(x, dtype=np.float32)
    w_attn = np.asarray(w_attn, dtype=np.float32)
    b_attn = np.asarray(b_attn, dtype=np.float32)
    masks = make_masks(dt_name)
    in_maps = []
    for c in range(G):
        b, g = c // 2, c % 2
        sq = slice(512 * g, 512 * g + 512)
        sk = slice(C + 512 * g, C + 512 * g + 512)
        sv = slice(2 * C + 512 * g, 2 * C + 512 * g + 512)
        wqkT = np.ascontiguousarray(
            np.concatenate([w_attn[sq], w_attn[sk]], axis=0).T.astype(np_dt))
        wvT = np.ascontiguousarray(w_attn[sv].T.astype(np_dt))
        wprojT = np.ascontiguousarray(
            np.asarray(w_proj, np.float32)[:, 512 * g:512 * g + 512].T.astype(np_dt))
        bqk = np.ascontiguousarray(
            np.concatenate([b_attn[sq], b_attn[sk]]).reshape(8, 128).T.astype(np.float32))
        bv = np.ascontiguousarray(b_attn[sv].reshape(1, 512).astype(np_dt))
        xT = np.ascontiguousarray(x[b].T.astype(np_dt))
        in_maps.append({
            "xT": xT, "wqkT": wqkT, "wvT": wvT, "wprojT": wprojT,
            "masks": masks, "bqk": bqk, "bv": bv,
        })
    return in_maps


def unshard(results, b_proj):
    out = np.empty((B, T, C), dtype=np.float32)
    for b in range(B):
        part = results[2 * b]["out"] + results[2 * b + 1]["out"]
        out[b] = part.T + np.asarray(b_proj, np.float32)[None, :]
    return out


def kernel(x, w_attn, b_attn, w_proj, b_proj):
    nc = build_nc(DT_NAME)
    in_maps = prep_inputs(x, w_attn, b_attn, w_proj, DT_NAME)
    res = run_bass_kernel_spmd(nc, in_maps, list(range(G)))
    return unshard(res.results, b_proj)


if __name__ == "__main__":
    rng = np.random.default_rng(0)
    x = rng.standard_normal((B, T, C), dtype=np.float32)
    w_attn = (rng.standard_normal((3 * C, C), dtype=np.float32) * 0.02)
    b_attn = np.zeros(3 * C, np.float32)
    w_proj = (rng.standard_normal((C, C), dtype=np.float32) * 0.02)
    b_proj = np.zeros(C, np.float32)
    out = kernel(x, w_attn, b_attn, w_proj, b_proj)
    print("out shape:", out.shape, out.dtype)


# revision 24
# speedup vs baseline: 313.7094x; 313.7094x over previous
"""Trainium2 Bass kernel for pair-masked causal self-attention.

Problem: B=4, T=2048, C=1024, H=16 heads (hd=64), GPT-style CausalSelfAttention
with a modified causal mask (odd query i cannot attend to i-1).

Sharding: 8 cores = 4 batches x 2 head-groups (8 heads each). No collectives:
each core computes a partial c_proj output (its 512 channels of y), partials
are summed pairwise on the host.

Per-core layout tricks:
- q,k are computed TRANSPOSED [c_out, t] (channel on partitions) so the
  scores matmul produces ST[k_pos, q_pos] directly (softmax reduction lands
  on the free dim of the AV matmul); two heads share the 128-row PE array
  via row-group packing (base partitions 0/64 -> concurrent on HW).
- v is computed in natural [t, c] orientation and stored per head as
  [ones(64) | v(64)], so one AV matmul yields the softmax denominator
  (rows 0-63, broadcast across 64 partitions) AND the y numerator
  (rows 64-127). Normalization is then 2 DVE ops per head:
  reciprocal_approx_fast straight from PSUM (base 0) and a mixed-base
  PSUM[64:128] x SBUF multiply into the yT tile.
- Causal + pair masking reduces to ONE static 128x128 mask applied to the
  diagonal sub-block of score tiles; fully-masked tiles are never computed
  and diagonal tiles are column-trimmed.
- Emission is software-pipelined: Tile schedules statically per engine, so
  qkv block n, attention block n-1, and (all) projection blocks are woven
  (Bresenham merge) to keep PE ~91%% busy; projections are piled into the
  ACT-bound final attention block.
"""

import numpy as np
import ml_dtypes

import concourse.bass as bass
import concourse.bacc as bacc
import concourse.tile as tile
from concourse import mybir
from concourse.bass_utils import run_bass_kernel_spmd

B, T, C, H = 4, 2048, 1024, 16
HD = C // H          # 64
G = 8                # cores
HPC = 8              # heads per core
PAIRS = HPC // 2     # head pairs per core
NT = T // 512        # 4 q/t column blocks of 512
KT = T // 128        # 16 k row tiles of 128
CT = C // 128        # 8 c_in tiles

DT_NAME = "bfloat16"   # "bfloat16" | "float32r" (f32r does not fit SBUF at this size)

_cache = {}


def _dt(dt_name):
    return getattr(mybir.dt, dt_name)


def _np_dt(dt_name):
    return np.float32 if dt_name == "float32r" else ml_dtypes.bfloat16


def build_nc(dt_name=DT_NAME):
    """Build (and cache) the per-core Bass program."""
    if dt_name in _cache:
        return _cache[dt_name]

    DT = _dt(dt_name)
    F32 = mybir.dt.float32
    nc = bacc.Bacc()

    xT_d = nc.declare_dram_parameter("xT", [C, T], DT, isOutput=False)
    wqk_d = nc.declare_dram_parameter("wqkT", [C, 1024], DT, isOutput=False)
    wv_d = nc.declare_dram_parameter("wvT", [C, 512], DT, isOutput=False)
    wp_d = nc.declare_dram_parameter("wprojT", [512, C], DT, isOutput=False)
    mask_d = nc.declare_dram_parameter("masks", [128, 256], DT, isOutput=False)
    bqk_d = nc.declare_dram_parameter("bqk", [128, 8], F32, isOutput=False)
    bv_d = nc.declare_dram_parameter("bv", [1, 512], DT, isOutput=False)
    out_d = nc.declare_dram_parameter("out", [C, T], F32, isOutput=True)

    EXP = mybir.ActivationFunctionType.Exp
    SCALE = 1.0 / float(np.sqrt(HD))

    with tile.TileContext(nc) as tc:
        with (
            tc.tile_pool(name="persist", bufs=1) as persist,
            tc.tile_pool(name="xw", bufs=1) as xw,
            tc.tile_pool(name="attnc", bufs=1) as attnc,
            tc.tile_pool(name="es_p", bufs=8) as es_p,
            tc.tile_pool(name="norm_p", bufs=3) as norm_p,
            tc.tile_pool(name="stage_p", bufs=4) as stage_p,
            tc.tile_pool(name="mm_ps", bufs=2, space="PSUM") as mm_ps,
            tc.tile_pool(name="st_ps", bufs=2, space="PSUM") as st_ps,
            tc.tile_pool(name="y_ps", bufs=2, space="PSUM") as y_ps,
        ):
            # ---- persistent tensors ----
            qkT = [persist.tile([128, T], DT, tag=f"qkT{m}", name=f"qkT{m}") for m in range(8)]
            v_aug = [persist.tile([128, PAIRS, 256], DT, tag=f"va{t}", name=f"va{t}") for t in range(KT)]
            yT = [persist.tile([128, T], DT, tag=f"yT{p}", name=f"yT{p}") for p in range(PAIRS)]

            # ---- input tiles ----
            xT = [xw.tile([128, T], DT, tag=f"xT{i}", name=f"xT{i}") for i in range(CT)]
            wqk = [xw.tile([128, 1024], DT, tag=f"wqk{i}", name=f"wqk{i}") for i in range(CT)]
            wv = [xw.tile([128, 512], DT, tag=f"wv{i}", name=f"wv{i}") for i in range(CT)]
            wp = [attnc.tile([128, C], DT, tag=f"wp{i}", name=f"wp{i}") for i in range(4)]
            msk = attnc.tile([128, 2, 128], DT, tag="msk", name="msk")
            bqk = attnc.tile([128, 8], F32, tag="bqk")
            bv = attnc.tile([1, 512], DT, tag="bv")
            ones_r = attnc.tile([1, 128], DT, tag="ones")

            nc.vector.memset(ones_r, 1.0)
            warm = attnc.tile([1, 1], DT, tag="warm")
            nc.scalar.activation(warm, ones_r[:, 0:1], EXP, scale=1.0)
            # column-sliced xT loads: block 0 only gates on its own slice
            for i in range(CT):
                nc.sync.dma_start(wqk[i], wqk_d[128 * i:128 * i + 128, :])
                nc.sync.dma_start(xT[i][:, 0:512], xT_d[128 * i:128 * i + 128, 0:512])
                if i == 1:
                    nc.sync.dma_start(bqk, bqk_d[:])
                    nc.sync.dma_start(bv, bv_d[:])
            for i in range(CT):
                nc.sync.dma_start(wv[i], wv_d[128 * i:128 * i + 128, :])
            nc.sync.dma_start(msk, mask_d.rearrange("p (h q) -> p h q", h=2))
            for i in range(CT):
                nc.sync.dma_start(xT[i][:, 512:T], xT_d[128 * i:128 * i + 128, 512:T])
            for i in range(4):
                nc.sync.dma_start(wp[i], wp_d[128 * i:128 * i + 128, :])

            # ---- software-pipelined emission ----
            # Tile produces a static per-engine order, so PE stalls unless
            # independent matmuls are woven between dependent ST->exp->AV
            # chains. Streams: qkv block n || attention j=n-1 || proj j=n-2.

            def qkv_psum(n, gi, nm):
                # block 0 runs before attention: borrow the idle st/y psum
                # slots so more accumulation groups stay open while input
                # DMAs stream in (otherwise 2 mm slots serialize startup)
                if n == 0:
                    r = gi % 3
                    if r == 1:
                        return st_ps.tile([128, 2, 512], F32, tag="st",
                                          name=nm)[:, 0, :]
                    if r == 2:
                        return y_ps.tile([128, 512], F32, tag="y", name=nm)
                return mm_ps.tile([128, 512], F32, tag="mm", name=nm)

            def qkv_groups(n):
                tsl = bass.ts(n, 512)
                groups = []
                for m in range(8):
                    half_state = {}
                    def g1(m=m, tsl=tsl, hs=half_state):
                        ps = qkv_psum(n, m, f"mm_qk_{n}_{m}")
                        hs['ps'] = ps
                        for kc in range(CT // 2):
                            nc.tensor.matmul(
                                ps, wqk[kc][:, 128 * m:128 * m + 128], xT[kc][:, tsl],
                                start=(kc == 0), stop=False)
                    def g2(m=m, tsl=tsl, hs=half_state):
                        ps = hs['ps']
                        for kc in range(CT // 2, CT):
                            nc.tensor.matmul(
                                ps, wqk[kc][:, 128 * m:128 * m + 128], xT[kc][:, tsl],
                                start=False, stop=(kc == CT - 1))
                        nc.vector.tensor_scalar_add(qkT[m][:, tsl], ps, bqk[:, m:m + 1])
                    groups.append(g1)
                    groups.append(g2)
                for tt in range(4 * n, 4 * n + 4):
                    def g(tt=tt):
                        ps = qkv_psum(n, 8 + tt - 4 * n, f"mm_v_{tt}")
                        for kc in range(CT):
                            nc.tensor.matmul(
                                ps, xT[kc][:, 128 * tt:128 * tt + 128], wv[kc],
                                start=(kc == 0), stop=False)
                        nc.tensor.matmul(ps, ones_r, bv, start=False, stop=True,
                                         skip_group_check=True)
                        psv = ps.rearrange("p (pr two d) -> p pr two d", pr=PAIRS, two=2)
                        nc.vector.memset(v_aug[tt][:, :, 0:64], 1.0)
                        nc.vector.memset(v_aug[tt][:, :, 128:192], 1.0)
                        nc.vector.tensor_copy(v_aug[tt][:, :, 64:128], psv[:, :, 0, :])
                        nc.vector.tensor_copy(v_aug[tt][:, :, 192:256], psv[:, :, 1, :])
                    groups.append(g)
                return groups

            def attn_units(j):
                qsl0 = 512 * j
                kk_hi = 4 * j + 4
                units = []
                ys = {}

                pend = {}

                def emit_av(p, kk, es, q0):
                    yA, yB = ys[p]
                    nc.tensor.matmul(yA[:, q0:512], v_aug[kk][:, p, 0:128],
                                     es[:, 0, q0:512],
                                     start=(kk == 0), stop=(kk == kk_hi - 1),
                                     skip_group_check=True)
                    nc.tensor.matmul(yB[:, q0:512], v_aug[kk][:, p, 128:256],
                                     es[:, 1, q0:512],
                                     start=(kk == 0), stop=(kk == kk_hi - 1),
                                     skip_group_check=True)

                def mk_kk(p, kk):
                    def u():
                        # software-pipeline: flush previous kk's AV first so
                        # its exp has had a full unit of slack
                        if p in pend:
                            emit_av(*pend.pop(p))
                        if kk == 0:
                            ys[p] = (y_ps.tile([128, 512], F32, tag="y", name=f"yA{j}_{p}"),
                                     y_ps.tile([128, 512], F32, tag="y", name=f"yB{j}_{p}"))
                        d = kk - 4 * j
                        q0 = 128 * d if d >= 0 else 0
                        st = st_ps.tile([128, 2, 512], F32, tag="st", name=f"st{j}_{p}_{kk}")
                        kT_t = qkT[4 + p]
                        qT_t = qkT[p]
                        ksl = bass.ts(kk, 128)
                        qsl = bass.ds(qsl0 + q0, 512 - q0)
                        nc.tensor.matmul(st[:, 0, q0:512],
                                         kT_t[0:64, ksl], qT_t[0:64, qsl],
                                         start=True, stop=True)
                        nc.tensor.matmul(st[:, 1, q0:512],
                                         kT_t[64:128, ksl], qT_t[64:128, qsl],
                                         start=True, stop=True)
                        es = es_p.tile([128, 2, 512], DT, tag="es", name=f"es{j}_{p}_{kk}")
                        nc.scalar.activation(es[:, :, q0:512], st[:, :, q0:512],
                                             EXP, scale=SCALE)
                        if d >= 0:
                            nc.vector.tensor_mul(es[:, :, q0:q0 + 128],
                                                 es[:, :, q0:q0 + 128], msk)
                        pend[p] = (p, kk, es, q0)
                    return u

                def mk_norm(p):
                    def u():
                        if p in pend:
                            emit_av(*pend.pop(p))
                        yA, yB = ys[p]
                        tsl2 = bass.ds(qsl0, 512)
                        recA = norm_p.tile([64, 512], F32, tag="rec", name=f"recA{j}_{p}")
                        nc.vector.reciprocal_approx_fast(out=recA, in_=yA[0:64, :])
                        nc.vector.tensor_mul(yT[p][0:64, tsl2], yA[64:128, :], recA)
                        recB = norm_p.tile([64, 512], F32, tag="rec", name=f"recB{j}_{p}")
                        nc.vector.reciprocal_approx_fast(out=recB, in_=yB[0:64, :])
                        nc.vector.tensor_mul(yT[p][64:128, tsl2], yB[64:128, :], recB)
                    return u

                for p in range(PAIRS):
                    for kk in range(kk_hi):
                        units.append(mk_kk(p, kk))
                    units.append(mk_norm(p))
                return units

            def proj_groups(j, borrow=False):
                qsl0 = 512 * j
                groups = []
                for o in range(8):
                    def g(o=o):
                        if borrow and o % 3 == 1:
                            pp = st_ps.tile([128, 2, 512], F32, tag="st",
                                            name=f"mm_pj_{j}_{o}")[:, 0, :]
                        elif borrow and o % 3 == 2:
                            pp = y_ps.tile([128, 512], F32, tag="y",
                                           name=f"mm_pj_{j}_{o}")
                        else:
                            pp = mm_ps.tile([128, 512], F32, tag="mm",
                                            name=f"mm_pj_{j}_{o}")
                        for cpt in range(4):
                            nc.tensor.matmul(pp, wp[cpt][:, 128 * o:128 * o + 128],
                                             yT[cpt][:, bass.ds(qsl0, 512)],
                                             start=(cpt == 0), stop=(cpt == 3))
                        stg = stage_p.tile([128, 512], F32, tag="stg", name=f"stg{j}_{o}")
                        nc.vector.tensor_copy(stg, pp)
                        nc.sync.dma_start(
                            out_d[128 * o:128 * o + 128, qsl0:qsl0 + 512], stg)
                    groups.append(g)
                return groups

            def weave(*streams):
                streams = [list(st_) for st_ in streams if st_]
                order = []
                for si, st_ in enumerate(streams):
                    for i, fn in enumerate(st_):
                        order.append(((i + 0.5) / len(st_), si, i, fn))
                order.sort(key=lambda t: (t[0], t[1]))
                for _, _, _, fn in order:
                    fn()

            for n in range(NT + 2):
                pg = []
                if n == NT:   # pile proj(0..2) into the ACT-bound attn(3) block
                    pg = proj_groups(0) + proj_groups(1) + proj_groups(2)
                elif n == NT + 1:
                    pg = proj_groups(3, borrow=True)
                weave(
                    qkv_groups(n) if n < NT else [],
                    attn_units(n - 1) if 1 <= n <= NT else [],
                    pg,
                )

    nc.compile()
    _cache[dt_name] = nc
    return nc


def make_masks(dt_name=DT_NAME):
    np_dt = _np_dt(dt_name)
    kk = np.arange(128)[:, None]
    qq = np.arange(128)[None, :]
    r = qq - kk
    m = ((r >= 0) & ~((r == 1) & (qq % 2 == 1))).astype(np_dt)
    masks = np.zeros((128, 256), dtype=np_dt)
    masks[:, 0:128] = m
    masks[:, 128:256] = m
    return masks


def prep_inputs(x, w_attn, b_attn, w_proj, dt_name=DT_NAME):
    np_dt = _np_dt(dt_name)
    x = np.asarray(x, dtype=np.float32)
    w_attn = np.asarray(w_attn, dtype=np.float32)
    b_attn = np.asarray(b_attn, dtype=np.float32)
    masks = make_masks(dt_name)
    in_maps = []
    for c in range(G):
        b, g = c // 2, c % 2
        sq = slice(512 * g, 512 * g + 512)
        sk = slice(C + 512 * g, C + 512 * g + 512)
        sv = slice(2 * C + 512 * g, 2 * C + 512 * g + 512)
        wqkT = np.ascontiguousarray(
            np.concatenate([w_attn[sq], w_attn[sk]], axis=0).T.astype(np_dt))
        wvT = np.ascontiguousarray(w_attn[sv].T.astype(np_dt))
        wprojT = np.ascontiguousarray(
            np.asarray(w_proj, np.float32)[:, 512 * g:512 * g + 512].T.astype(np_dt))
        bqk = np.ascontiguousarray(
            np.concatenate([b_attn[sq], b_attn[sk]]).reshape(8, 128).T.astype(np.float32))
        bv = np.ascontiguousarray(b_attn[sv].reshape(1, 512).astype(np_dt))
        xT = np.ascontiguousarray(x[b].T.astype(np_dt))
        in_maps.append({
            "xT": xT, "wqkT": wqkT, "wvT": wvT, "wprojT": wprojT,
            "masks": masks, "bqk": bqk, "bv": bv,
        })
    return in_maps


def unshard(results, b_proj):
    out = np.empty((B, T, C), dtype=np.float32)
    for b in range(B):
        part = results[2 * b]["out"] + results[2 * b + 1]["out"]
        out[b] = part.T + np.asarray(b_proj, np.float32)[None, :]
    return out


def kernel(x, w_attn, b_attn, w_proj, b_proj):
    nc = build_nc(DT_NAME)
    in_maps = prep_inputs(x, w_attn, b_attn, w_proj, DT_NAME)
    res = run_bass_kernel_spmd(nc, in_maps, list(range(G)))
    return unshard(res.results, b_proj)


if __name__ == "__main__":
    rng = np.random.default_rng(0)
    x = rng.standard_normal((B, T, C), dtype=np.float32)
    w_attn = (rng.standard_normal((3 * C, C), dtype=np.float32) * 0.02)
    b_attn = np.zeros(3 * C, np.float32)
    w_proj = (rng.standard_normal((C, C), dtype=np.float32) * 0.02)
    b_proj = np.zeros(C, np.float32)
    out = kernel(x, w_attn, b_attn, w_proj, b_proj)
    print("out shape:", out.shape, out.dtype)
